# revision 63
# baseline (speedup 1.0000x reference)
"""BANLayer Trainium2 kernel.

Data-parallel over batch: 8 examples -> 8 NeuronCores, weights replicated.

The host wrapper does layout-only prep (transposes / fp16 casts / bias-vector
reshapes/broadcasts — no FLOPs on activations beyond dtype rounding); the
device does all the math:

Per-core (one example; i=num_v=128, j=num_q=512, k=in_dim=128,
hd=H_OUT*H_DIM=512; chunk c in 0..3 <-> (h, dhalf)):

  v_T[hd, i]  = Wv.T @ v.T (+bv)            (4 MMs, N=128)
  q_T[hd, j]  = Wq.T @ q.T                  (4 MMs, N=512)
  qs_T        = q_T * wa + (bq*wa)          (per-partition scale+bias, fp16)
  S_T[j, i]   = qs_T.T @ v_T  per head      (16 MMs; softmax dim = free dim;
                                             no max-subtraction: |S|<~0.2 and
                                             the +ba shift cancels in softmax)
  E_T         = exp(S_T)                    (1 big ACT op per head)
  colsum, r   = DVE 3D reduce + reciprocal  (per head)
  E_sc        = E_T * r                     (= att transposed; fp16; DMA'd out
                                             per head, un-transposed on host)
  q_nat[j,hd] = q @ Wq + bq                 (4 MMs, scheduled into the softmax
                                             latency; bias added during psum
                                             evacuation vs a host-broadcast bq)
  ctx_T[d, i] = q_nat.T @ E_sc              (16 MMs; deferred normalization in E_sc)
  head[hd]    = sum_i v_T * ctx_T           (DVE fused mult + free-dim accum)
  fusedT[o]   = Wo.T @ head (+bo)           (8 tiny N=1 MMs; output transposed,
                                             un-transposed on host)

fp16 matmul operands, fp32 accumulation/softmax/outputs.
"""

import numpy as np

H_OUT = 2
H_DIM = 256
NV = 128
NQ = 512
KD = 128          # V_DIM == Q_DIM
HD = H_OUT * H_DIM
N_CORES = 8

# packed fp16 input layout: [128, PK_COLS]
PK_WQ = 0                     # Wq16            [128, 512]
PK_QT = PK_WQ + HD            # qT16            [128, 512]
PK_WV = PK_QT + NQ            # Wv16            [128, 512]
PK_VT = PK_WV + HD            # vT16            [128, 128]
PK_WO = PK_VT + NV            # Wo16 (chunked)  [128, 1024]
PK_BQ = PK_WO + 4 * H_DIM     # bq16 broadcast  [128, 512]
PK_COLS = PK_BQ + HD          # 3200
PK_A1 = PK_WV                 # first DMA: q-side
PK_A2 = PK_WO                 # second DMA: v-side

_CACHE = {}


def _build(stage=99):
    import concourse.bacc as bacc
    import concourse.tile as tile
    from concourse import mybir

    F32 = mybir.dt.float32
    F16 = mybir.dt.float16
    Identity = mybir.ActivationFunctionType.Identity
    Exp = mybir.ActivationFunctionType.Exp
    MULT = mybir.AluOpType.mult
    ADD = mybir.AluOpType.add
    BYPASS = mybir.AluOpType.bypass

    nc = bacc.Bacc("TRN2", target_bir_lowering=False, debug=False)

    # CT32 cols: 0-3 bv_T, 4-7 (bq*wa)_T, 8-11 wa_T4, 12-13 bo_T
    ct_d = nc.dram_tensor("CT32", [128, 14], F32, kind="ExternalInput")
    pk_d = nc.dram_tensor("PK16", [128, PK_COLS], F16, kind="ExternalInput")

    fusedT_d = nc.dram_tensor("fusedT", [128, 2], F32, kind="ExternalOutput")
    attT_d = nc.dram_tensor("attT16", [128, 8 * 128], F16, kind="ExternalOutput")

    def emit(consts, work, pbig, pquad, pfused):
        # ------------- loads (q-side first, then v-side, then the rest) --------
        pk = work.tile([128, PK_COLS], F16)
        nc.sync.dma_start(out=pk[:, :PK_A1], in_=pk_d[:, :PK_A1])
        CT = work.tile([128, 14], F32)
        nc.scalar.dma_start(out=CT[:], in_=ct_d[:])
        nc.sync.dma_start(out=pk[:, PK_A1:PK_A2], in_=pk_d[:, PK_A1:PK_A2])
        nc.sync.dma_start(out=pk[:, PK_A2:], in_=pk_d[:, PK_A2:])

        Wv16 = pk[:, PK_WV : PK_WV + HD]
        vT16 = pk[:, PK_VT : PK_VT + NV]
        Wq16 = pk[:, PK_WQ : PK_WQ + HD]
        qT16 = pk[:, PK_QT : PK_QT + NQ]
        Wo16 = pk[:, PK_WO : PK_WO + 4 * H_DIM]
        bqb16 = pk[:, PK_BQ : PK_BQ + HD]

        zeros1 = consts.tile([128, 1], F32)
        nc.vector.memset(zeros1[:], 0.0)

        # PE warm-up: the HAM clock-gate needs ~3.4us of sustained PE activity
        # to unthrottle 1.2GHz -> 2.4GHz. The PE is otherwise idle while the
        # input DMAs land, so burn that window on dummy matmuls against a
        # const tile; the real matmul stream then starts at full clock.
        dum16 = consts.tile([128, 128], F16)
        nc.vector.memset(dum16[:], 0.0)
        dp = pfused.tile([128, 2], F32, tag="fp", name="dp")
        for _ in range(22):
            nc.tensor.matmul(dp[:], dum16[:], dum16[:, 0:2])

        qs_T = work.tile([128, 4 * NQ], F16)   # [d, c, j]
        v_T16 = work.tile([128, 4 * NV], F16)  # [d, c, i]
        vp = pquad.tile([128, NQ], F32, tag="quad")

        def qT_chunk(c):
            qp = pbig.tile([128, NQ], F32, tag="big", name=f"qp{c}")
            nc.tensor.matmul(qp[:], Wq16[:, c * 128 : (c + 1) * 128], qT16)
            if c % 2 == 0:
                nc.vector.tensor_scalar(
                    out=qs_T[:, c * NQ : (c + 1) * NQ],
                    in0=qp[:],
                    scalar1=CT[:, 8 + c : 9 + c],
                    scalar2=CT[:, 4 + c : 5 + c],
                    op0=MULT,
                    op1=ADD,
                )
            else:
                nc.scalar.activation(
                    qs_T[:, c * NQ : (c + 1) * NQ],
                    qp[:],
                    Identity,
                    bias=CT[:, 4 + c : 5 + c],
                    scale=CT[:, 8 + c : 9 + c],
                )

        # critical path first: q_T chunks for head 0, then v_T, then S_h0
        qT_chunk(0)
        qT_chunk(1)
        for c in range(4):
            nc.tensor.matmul(
                vp[:, c * 128 : (c + 1) * 128], Wv16[:, c * 128 : (c + 1) * 128], vT16
            )
        for c in range(4):
            nc.vector.tensor_scalar(
                out=v_T16[:, c * NV : (c + 1) * NV],
                in0=vp[:, c * 128 : (c + 1) * 128],
                scalar1=CT[:, c : c + 1],
                scalar2=None,
                op0=ADD,
            )

        if stage <= 1:
            return

        # ---------------- score head 0 ----------------
        E_T16 = work.tile([128, 8 * 128], F16)  # [j, (h,jc), i]
        E_sc16 = work.tile([128, 8 * 128], F16)
        colsT = work.tile([128, 8], F32)
        r_T = work.tile([128, 8], F32)
        AXX = mybir.AxisListType.X
        sps = []

        def score_mms(h):
            sp = pquad.tile([128, NQ], F32, tag="quad")
            sps.append(sp)
            for jc in range(4):
                for dc in range(2):
                    c = h * 2 + dc
                    nc.tensor.matmul(
                        sp[:, jc * 128 : (jc + 1) * 128],
                        qs_T[:, c * NQ + jc * 128 : c * NQ + (jc + 1) * 128],
                        v_T16[:, c * NV : (c + 1) * NV],
                        start=(dc == 0),
                        stop=(dc == 1),
                    )

        def softmax(h):
            # pipelined per 2-jc half: exp -> reduce -> recip -> scale, so the
            # first E_sc chunks are ready ~1us after the first S chunks stop
            sp = sps[h]
            for half in range(2):
                base = h * 512 + half * 256
                cb = h * 4 + half * 2
                nc.scalar.activation(
                    E_T16[:, base : base + 256],
                    sp[:, half * 256 : (half + 1) * 256],
                    Exp,
                    bias=zeros1[:],
                )
                nc.vector.reduce_sum(
                    out=colsT[:, cb : cb + 2],
                    in_=E_T16[:, base : base + 256].rearrange(
                        "p (a b) -> p a b", b=128
                    ),
                    axis=AXX,
                )
                nc.vector.reciprocal(r_T[:, cb : cb + 2], colsT[:, cb : cb + 2])
                for k in range(2):
                    hc = cb + k
                    if k == 0:
                        nc.vector.tensor_scalar(
                            out=E_sc16[:, hc * 128 : (hc + 1) * 128],
                            in0=E_T16[:, hc * 128 : (hc + 1) * 128],
                            scalar1=r_T[:, hc : hc + 1],
                            scalar2=None,
                            op0=MULT,
                        )
                    else:
                        nc.scalar.activation(
                            E_sc16[:, hc * 128 : (hc + 1) * 128],
                            E_T16[:, hc * 128 : (hc + 1) * 128],
                            Identity,
                            bias=zeros1[:],
                            scale=r_T[:, hc : hc + 1],
                        )
            nc.sync.dma_start(
                out=attT_d[:, h * 512 : (h + 1) * 512],
                in_=E_sc16[:, h * 512 : (h + 1) * 512],
            )

        score_mms(0)
        qT_chunk(2)
        qT_chunk(3)
        softmax(0)

        score_mms(1)

        # ---------------- q_nat: PE filler during softmax latency ------
        # evacuation split: DVE adds the bias in-pass; ACT evacuations get the
        # bias via a K=1 ones x bq matmul into the psum group instead
        # bias-add evacuation split per head-half: finer DVE ops block the
        # softmax chain less, and ctx head 0 only waits for the h0 halves
        q_nat16 = work.tile([128, 4 * HD], F16)  # [j, jc, hd]
        for jc in range(4):
            qnp = pbig.tile([128, HD], F32, tag="big", name=f"qnp{jc}")
            nc.tensor.matmul(qnp[:], qT16[:, jc * 128 : (jc + 1) * 128], Wq16)
            nc.vector.tensor_tensor(
                out=q_nat16[:, jc * HD : (jc + 1) * HD],
                in0=qnp[:],
                in1=bqb16,
                op=ADD,
            )

        softmax(1)

        if stage <= 2:
            return

        # ---------------- ctx (transposed) + head + fusedT (interleaved) ------
        # two static psum tiles, alternating by chunk parity: the DVE head
        # reduction of chunk c overlaps PE writing chunk c+1 in the other bank
        ctxA = pquad.tile([128, 2, 128], F32, tag="ctxa", bufs=1)  # chunks 0, 2
        ctxB = pquad.tile([128, 2, 128], F32, tag="ctxb", bufs=1)  # chunks 1, 3
        headT = work.tile([128, 4], F32)
        headT16 = work.tile([128, 4], F16)
        prod = work.tile([128, 128], F32)
        fp = pfused.tile([128, 2], F32, tag="fp")

        def _ctx_slice(c):
            t = ctxA if c % 2 == 0 else ctxB
            return t[:, c // 2, :]

        def ctx_mms(c):
            h = c // 2
            for jc in range(4):
                nc.tensor.matmul(
                    _ctx_slice(c),
                    q_nat16[:, jc * HD + c * 128 : jc * HD + (c + 1) * 128],
                    E_sc16[:, (h * 4 + jc) * 128 : (h * 4 + jc + 1) * 128],
                    start=(jc == 0),
                    stop=(jc == 3),
                )

        def head_chunk(c):
            nc.vector.scalar_tensor_tensor(
                out=prod[:],
                in0=_ctx_slice(c),
                scalar=1.0,
                in1=v_T16[:, c * NV : (c + 1) * NV],
                op0=BYPASS,
                op1=MULT,
                accum_out=headT[:, c : c + 1],
            )
            nc.vector.tensor_copy(headT16[:, c : c + 1], headT[:, c : c + 1])

        def fused_mms(oh):
            # fusedT[oh*128+o, 1] = sum_c Wo[c-chunk, oh-half].T @ headT16[:, c]
            for c in range(4):
                nc.tensor.matmul(
                    fp[:, oh : oh + 1],
                    Wo16[:, c * H_DIM + oh * 128 : c * H_DIM + (oh + 1) * 128],
                    headT16[:, c : c + 1],
                    start=(c == 0),
                    stop=(c == 3),
                )

        ctx_mms(0)
        ctx_mms(1)
        head_chunk(0)
        ctx_mms(2)
        head_chunk(1)
        ctx_mms(3)
        head_chunk(2)
        head_chunk(3)
        fused_mms(0)
        fused_mms(1)

        fused_sb = work.tile([128, 2], F32)
        nc.vector.tensor_tensor(out=fused_sb[:], in0=fp[:], in1=CT[:, 12:14], op=ADD)
        nc.sync.dma_start(out=fusedT_d[:], in_=fused_sb[:])

    with tile.TileContext(nc) as tc:
        with (
            tc.tile_pool(name="consts", bufs=1) as consts,
            tc.tile_pool(name="work", bufs=1) as work,
            tc.tile_pool(name="pbig", bufs=3, space="PSUM") as pbig,
            tc.tile_pool(name="pquad", bufs=2, space="PSUM") as pquad,
            tc.tile_pool(name="pfused", bufs=1, space="PSUM") as pfused,
        ):
            emit(consts, work, pbig, pquad, pfused)

    nc.compile()
    return nc


def get_nc(stage=99):
    key = ("nc", stage)
    if key not in _CACHE:
        _CACHE[key] = _build(stage)
    return _CACHE[key]


def prep_weights(Wv, bv, Wq, bq, wa, Wo, bo):
    """Host-side layout prep of the replicated weights (pure layout/dtype)."""
    f16, f32 = np.float16, np.float32
    Wq16 = np.asarray(Wq, f32).astype(f16)
    Wv16 = np.asarray(Wv, f32).astype(f16)
    Wo = np.asarray(Wo, f32)
    Wo16 = (
        np.transpose(Wo.reshape(4, 128, H_DIM), (1, 0, 2))
        .reshape(128, 4 * H_DIM)
        .astype(f16)
    )
    bq16 = np.asarray(bq, f32).reshape(1, HD).astype(f16)
    bqb16 = np.broadcast_to(bq16, (128, HD))
    bv_T = np.asarray(bv, f32).reshape(4, 128).T            # [128, 4]
    bq_T = np.asarray(bq, f32).reshape(4, 128).T            # [128, 4]
    wa_T = np.asarray(wa, f32).reshape(2, 128).T            # [128, 2]
    wa_T4 = np.concatenate([wa_T, wa_T], axis=1)            # [128, 4]
    bo_T = np.asarray(bo, f32).reshape(2, 128).T            # [128, 2]
    CT32 = np.ascontiguousarray(
        np.concatenate([bv_T, bq_T * wa_T4, wa_T4, bo_T], axis=1).astype(f32)
    )
    return {"CT32": CT32}, Wq16, Wv16, Wo16, bqb16


def prep_example(Wq16, Wv16, Wo16, bqb16, v_b, q_b):
    """Pack per-core fp16 operands (weights replicated + this example's v/q)."""
    f16 = np.float16
    vT16 = np.ascontiguousarray(np.asarray(v_b, np.float32).T).astype(f16)
    qT16 = np.ascontiguousarray(np.asarray(q_b, np.float32).T).astype(f16)
    pk = np.concatenate([Wq16, qT16, Wv16, vT16, Wo16, bqb16], axis=1)
    assert pk.shape == (128, PK_COLS)
    return {"PK16": np.ascontiguousarray(pk)}


def unprep_att(attT16):
    """[128 j, 8 (h,jc), 128 i] fp16 -> [2, 128 i, 512 j] fp32."""
    a = np.asarray(attT16).reshape(128, 2, 4, 128).astype(np.float32)
    return np.transpose(a, (1, 3, 2, 0)).reshape(H_OUT, NV, NQ)


def make_in_maps(v, q, Wv, bv, Wq, bq, wa, Wo, bo):
    common, Wq16, Wv16, Wo16, bqb16 = prep_weights(Wv, bv, Wq, bq, wa, Wo, bo)
    B = np.asarray(v).shape[0]
    return [
        dict(common, **prep_example(Wq16, Wv16, Wo16, bqb16, v[b], q[b]))
        for b in range(B)
    ]


def kernel(v, q, Wv, bv, Wq, bq, wa, ba, Wo, bo):
    from concourse.bass_utils import run_bass_kernel_spmd

    nc = get_nc()
    B = np.asarray(v).shape[0]
    assert B == N_CORES, f"expected batch {N_CORES}, got {B}"
    in_maps = make_in_maps(v, q, Wv, bv, Wq, bq, wa, Wo, bo)
    res = run_bass_kernel_spmd(nc, in_maps, core_ids=list(range(N_CORES)))
    fused = np.stack(
        [r["fusedT"].T.reshape(H_DIM) for r in res.results], axis=0
    )
    att = np.stack([unprep_att(r["attT16"]) for r in res.results], axis=0)
    att = att.reshape(B, H_OUT, NV * NQ, 1)
    return fused, att


# revision 64
# speedup vs baseline: 1.0799x; 1.0799x over previous
"""BANLayer Trainium2 kernel.

Data-parallel over batch: 8 examples -> 8 NeuronCores, weights replicated.

The host wrapper does layout-only prep (transposes / fp16 casts / bias-vector
reshapes/broadcasts — no FLOPs on activations beyond dtype rounding); the
device does all the math:

Per-core (one example; i=num_v=128, j=num_q=512, k=in_dim=128,
hd=H_OUT*H_DIM=512; chunk c in 0..3 <-> (h, dhalf)):

  v_T[hd, i]  = Wv.T @ v.T (+bv)            (4 MMs, N=128)
  q_T[hd, j]  = Wq.T @ q.T                  (4 MMs, N=512)
  qs_T        = q_T * wa + (bq*wa)          (per-partition scale+bias, fp16)
  S_T[j, i]   = qs_T.T @ v_T  per head      (16 MMs; softmax dim = free dim;
                                             no max-subtraction: |S|<~0.2 and
                                             the +ba shift cancels in softmax)
  E_T         = exp(S_T)                    (1 big ACT op per head)
  colsum, r   = DVE 3D reduce + reciprocal  (per head)
  E_sc        = E_T * r                     (= att transposed; fp16; DMA'd out
                                             per head, un-transposed on host)
  q_nat[j,hd] = q @ Wq + bq                 (4 MMs, scheduled into the softmax
                                             latency; bias added during psum
                                             evacuation vs a host-broadcast bq)
  ctx_T[d, i] = q_nat.T @ E_sc              (16 MMs; deferred normalization in E_sc)
  head[hd]    = sum_i v_T * ctx_T           (DVE fused mult + free-dim accum)
  fusedT[o]   = Wo.T @ head (+bo)           (8 tiny N=1 MMs; output transposed,
                                             un-transposed on host)

fp16 matmul operands, fp32 accumulation/softmax/outputs.
"""

import numpy as np

H_OUT = 2
H_DIM = 256
NV = 128
NQ = 512
KD = 128          # V_DIM == Q_DIM
HD = H_OUT * H_DIM
N_CORES = 8

# packed fp16 input layout: [128, PK_COLS]
PK_WQ = 0                     # Wq16            [128, 512]
PK_QT = PK_WQ + HD            # qT16            [128, 512]
PK_WV = PK_QT + NQ            # Wv16            [128, 512]
PK_VT = PK_WV + HD            # vT16            [128, 128]
PK_WO = PK_VT + NV            # Wo16 (chunked)  [128, 1024]
PK_BQ = PK_WO + 4 * H_DIM     # bq16 broadcast  [128, 512]
PK_COLS = PK_BQ + HD          # 3200
PK_A1 = PK_WV                 # first DMA: q-side
PK_A2 = PK_WO                 # second DMA: v-side

_CACHE = {}


def _build(stage=99):
    import concourse.bacc as bacc
    import concourse.tile as tile
    from concourse import mybir

    F32 = mybir.dt.float32
    F16 = mybir.dt.float16
    Identity = mybir.ActivationFunctionType.Identity
    Exp = mybir.ActivationFunctionType.Exp
    MULT = mybir.AluOpType.mult
    ADD = mybir.AluOpType.add
    BYPASS = mybir.AluOpType.bypass

    nc = bacc.Bacc("TRN2", target_bir_lowering=False, debug=False)

    # CT32 cols: 0-3 bv_T, 4-7 (bq*wa)_T, 8-11 wa_T4, 12-13 bo_T
    ct_d = nc.dram_tensor("CT32", [128, 14], F32, kind="ExternalInput")
    pk_d = nc.dram_tensor("PK16", [128, PK_COLS], F16, kind="ExternalInput")

    fusedT_d = nc.dram_tensor("fusedT", [128, 2], F32, kind="ExternalOutput")
    attT_d = nc.dram_tensor("attT16", [128, 8 * 128], F16, kind="ExternalOutput")

    def emit(consts, work, pbig, pquad, pfused):
        # ------------- loads (q-side first, then v-side, then the rest) --------
        pk = work.tile([128, PK_COLS], F16)
        nc.sync.dma_start(out=pk[:, :PK_A1], in_=pk_d[:, :PK_A1])
        CT = work.tile([128, 14], F32)
        nc.scalar.dma_start(out=CT[:], in_=ct_d[:])
        nc.sync.dma_start(out=pk[:, PK_A1:PK_A2], in_=pk_d[:, PK_A1:PK_A2])
        nc.sync.dma_start(out=pk[:, PK_A2:], in_=pk_d[:, PK_A2:])

        Wv16 = pk[:, PK_WV : PK_WV + HD]
        vT16 = pk[:, PK_VT : PK_VT + NV]
        Wq16 = pk[:, PK_WQ : PK_WQ + HD]
        qT16 = pk[:, PK_QT : PK_QT + NQ]
        Wo16 = pk[:, PK_WO : PK_WO + 4 * H_DIM]
        bqb16 = pk[:, PK_BQ : PK_BQ + HD]

        zeros1 = consts.tile([128, 1], F32)
        nc.vector.memset(zeros1[:], 0.0)

        qs_T = work.tile([128, 4 * NQ], F16)   # [d, c, j]
        v_T16 = work.tile([128, 4 * NV], F16)  # [d, c, i]
        vp = pquad.tile([128, NQ], F32, tag="quad")

        def qT_chunk(c):
            qp = pbig.tile([128, NQ], F32, tag="big", name=f"qp{c}")
            nc.tensor.matmul(qp[:], Wq16[:, c * 128 : (c + 1) * 128], qT16)
            if c % 2 == 0:
                nc.vector.tensor_scalar(
                    out=qs_T[:, c * NQ : (c + 1) * NQ],
                    in0=qp[:],
                    scalar1=CT[:, 8 + c : 9 + c],
                    scalar2=CT[:, 4 + c : 5 + c],
                    op0=MULT,
                    op1=ADD,
                )
            else:
                nc.scalar.activation(
                    qs_T[:, c * NQ : (c + 1) * NQ],
                    qp[:],
                    Identity,
                    bias=CT[:, 4 + c : 5 + c],
                    scale=CT[:, 8 + c : 9 + c],
                )

        # critical path first: q_T chunks for head 0, then v_T, then S_h0
        qT_chunk(0)
        qT_chunk(1)
        for c in range(4):
            nc.tensor.matmul(
                vp[:, c * 128 : (c + 1) * 128], Wv16[:, c * 128 : (c + 1) * 128], vT16
            )
        for c in range(4):
            nc.vector.tensor_scalar(
                out=v_T16[:, c * NV : (c + 1) * NV],
                in0=vp[:, c * 128 : (c + 1) * 128],
                scalar1=CT[:, c : c + 1],
                scalar2=None,
                op0=ADD,
            )

        if stage <= 1:
            return

        # ---------------- score head 0 ----------------
        E_T16 = work.tile([128, 8 * 128], F16)  # [j, (h,jc), i]
        E_sc16 = work.tile([128, 8 * 128], F16)
        colsT = work.tile([128, 8], F32)
        r_T = work.tile([128, 8], F32)
        AXX = mybir.AxisListType.X
        sps = []

        def score_mms(h):
            sp = pquad.tile([128, NQ], F32, tag="quad")
            sps.append(sp)
            for jc in range(4):
                for dc in range(2):
                    c = h * 2 + dc
                    nc.tensor.matmul(
                        sp[:, jc * 128 : (jc + 1) * 128],
                        qs_T[:, c * NQ + jc * 128 : c * NQ + (jc + 1) * 128],
                        v_T16[:, c * NV : (c + 1) * NV],
                        start=(dc == 0),
                        stop=(dc == 1),
                    )

        def softmax(h):
            # pipelined per 2-jc half: exp -> reduce -> recip -> scale, so the
            # first E_sc chunks are ready ~1us after the first S chunks stop
            sp = sps[h]
            for half in range(2):
                base = h * 512 + half * 256
                cb = h * 4 + half * 2
                nc.scalar.activation(
                    E_T16[:, base : base + 256],
                    sp[:, half * 256 : (half + 1) * 256],
                    Exp,
                    bias=zeros1[:],
                )
                nc.vector.reduce_sum(
                    out=colsT[:, cb : cb + 2],
                    in_=E_T16[:, base : base + 256].rearrange(
                        "p (a b) -> p a b", b=128
                    ),
                    axis=AXX,
                )
                nc.vector.reciprocal(r_T[:, cb : cb + 2], colsT[:, cb : cb + 2])
                for k in range(2):
                    hc = cb + k
                    if k == 0:
                        nc.vector.tensor_scalar(
                            out=E_sc16[:, hc * 128 : (hc + 1) * 128],
                            in0=E_T16[:, hc * 128 : (hc + 1) * 128],
                            scalar1=r_T[:, hc : hc + 1],
                            scalar2=None,
                            op0=MULT,
                        )
                    else:
                        nc.scalar.activation(
                            E_sc16[:, hc * 128 : (hc + 1) * 128],
                            E_T16[:, hc * 128 : (hc + 1) * 128],
                            Identity,
                            bias=zeros1[:],
                            scale=r_T[:, hc : hc + 1],
                        )
            nc.sync.dma_start(
                out=attT_d[:, h * 512 : (h + 1) * 512],
                in_=E_sc16[:, h * 512 : (h + 1) * 512],
            )

        score_mms(0)
        qT_chunk(2)
        qT_chunk(3)
        softmax(0)

        score_mms(1)

        # ---------------- q_nat: PE filler during softmax latency ------
        # evacuation split: DVE adds the bias in-pass; ACT evacuations get the
        # bias via a K=1 ones x bq matmul into the psum group instead
        # bias-add evacuation split per head-half: finer DVE ops block the
        # softmax chain less, and ctx head 0 only waits for the h0 halves
        q_nat16 = work.tile([128, 4 * HD], F16)  # [j, jc, hd]
        for jc in range(4):
            qnp = pbig.tile([128, HD], F32, tag="big", name=f"qnp{jc}")
            nc.tensor.matmul(qnp[:], qT16[:, jc * 128 : (jc + 1) * 128], Wq16)
            nc.vector.tensor_tensor(
                out=q_nat16[:, jc * HD : (jc + 1) * HD],
                in0=qnp[:],
                in1=bqb16,
                op=ADD,
            )

        softmax(1)

        if stage <= 2:
            return

        # ---------------- ctx (transposed) + head + fusedT (interleaved) ------
        # two static psum tiles, alternating by chunk parity: the DVE head
        # reduction of chunk c overlaps PE writing chunk c+1 in the other bank
        ctxA = pquad.tile([128, 2, 128], F32, tag="ctxa", bufs=1)  # chunks 0, 2
        ctxB = pquad.tile([128, 2, 128], F32, tag="ctxb", bufs=1)  # chunks 1, 3
        headT = work.tile([128, 4], F32)
        headT16 = work.tile([128, 4], F16)
        prod = work.tile([128, 128], F32)
        fp = pfused.tile([128, 2], F32)

        def _ctx_slice(c):
            t = ctxA if c % 2 == 0 else ctxB
            return t[:, c // 2, :]

        def ctx_mms(c):
            h = c // 2
            for jc in range(4):
                nc.tensor.matmul(
                    _ctx_slice(c),
                    q_nat16[:, jc * HD + c * 128 : jc * HD + (c + 1) * 128],
                    E_sc16[:, (h * 4 + jc) * 128 : (h * 4 + jc + 1) * 128],
                    start=(jc == 0),
                    stop=(jc == 3),
                )

        def head_chunk(c):
            nc.vector.scalar_tensor_tensor(
                out=prod[:],
                in0=_ctx_slice(c),
                scalar=1.0,
                in1=v_T16[:, c * NV : (c + 1) * NV],
                op0=BYPASS,
                op1=MULT,
                accum_out=headT[:, c : c + 1],
            )
            nc.vector.tensor_copy(headT16[:, c : c + 1], headT[:, c : c + 1])

        def fused_mms(oh):
            # fusedT[oh*128+o, 1] = sum_c Wo[c-chunk, oh-half].T @ headT16[:, c]
            for c in range(4):
                nc.tensor.matmul(
                    fp[:, oh : oh + 1],
                    Wo16[:, c * H_DIM + oh * 128 : c * H_DIM + (oh + 1) * 128],
                    headT16[:, c : c + 1],
                    start=(c == 0),
                    stop=(c == 3),
                )

        ctx_mms(0)
        ctx_mms(1)
        head_chunk(0)
        ctx_mms(2)
        head_chunk(1)
        ctx_mms(3)
        head_chunk(2)
        head_chunk(3)
        fused_mms(0)
        fused_mms(1)

        fused_sb = work.tile([128, 2], F32)
        nc.vector.tensor_tensor(out=fused_sb[:], in0=fp[:], in1=CT[:, 12:14], op=ADD)
        nc.sync.dma_start(out=fusedT_d[:], in_=fused_sb[:])

    with tile.TileContext(nc) as tc:
        with (
            tc.tile_pool(name="consts", bufs=1) as consts,
            tc.tile_pool(name="work", bufs=1) as work,
            tc.tile_pool(name="pbig", bufs=3, space="PSUM") as pbig,
            tc.tile_pool(name="pquad", bufs=2, space="PSUM") as pquad,
            tc.tile_pool(name="pfused", bufs=1, space="PSUM") as pfused,
        ):
            emit(consts, work, pbig, pquad, pfused)

    nc.compile()
    return nc


def get_nc(stage=99):
    key = ("nc", stage)
    if key not in _CACHE:
        _CACHE[key] = _build(stage)
    return _CACHE[key]


def prep_weights(Wv, bv, Wq, bq, wa, Wo, bo):
    """Host-side layout prep of the replicated weights (pure layout/dtype)."""
    f16, f32 = np.float16, np.float32
    Wq16 = np.asarray(Wq, f32).astype(f16)
    Wv16 = np.asarray(Wv, f32).astype(f16)
    Wo = np.asarray(Wo, f32)
    Wo16 = (
        np.transpose(Wo.reshape(4, 128, H_DIM), (1, 0, 2))
        .reshape(128, 4 * H_DIM)
        .astype(f16)
    )
    bq16 = np.asarray(bq, f32).reshape(1, HD).astype(f16)
    bqb16 = np.broadcast_to(bq16, (128, HD))
    bv_T = np.asarray(bv, f32).reshape(4, 128).T            # [128, 4]
    bq_T = np.asarray(bq, f32).reshape(4, 128).T            # [128, 4]
    wa_T = np.asarray(wa, f32).reshape(2, 128).T            # [128, 2]
    wa_T4 = np.concatenate([wa_T, wa_T], axis=1)            # [128, 4]
    bo_T = np.asarray(bo, f32).reshape(2, 128).T            # [128, 2]
    CT32 = np.ascontiguousarray(
        np.concatenate([bv_T, bq_T * wa_T4, wa_T4, bo_T], axis=1).astype(f32)
    )
    return {"CT32": CT32}, Wq16, Wv16, Wo16, bqb16


def prep_example(Wq16, Wv16, Wo16, bqb16, v_b, q_b):
    """Pack per-core fp16 operands (weights replicated + this example's v/q)."""
    f16 = np.float16
    vT16 = np.ascontiguousarray(np.asarray(v_b, np.float32).T).astype(f16)
    qT16 = np.ascontiguousarray(np.asarray(q_b, np.float32).T).astype(f16)
    pk = np.concatenate([Wq16, qT16, Wv16, vT16, Wo16, bqb16], axis=1)
    assert pk.shape == (128, PK_COLS)
    return {"PK16": np.ascontiguousarray(pk)}


def unprep_att(attT16):
    """[128 j, 8 (h,jc), 128 i] fp16 -> [2, 128 i, 512 j] fp32."""
    a = np.asarray(attT16).reshape(128, 2, 4, 128).astype(np.float32)
    return np.transpose(a, (1, 3, 2, 0)).reshape(H_OUT, NV, NQ)


def make_in_maps(v, q, Wv, bv, Wq, bq, wa, Wo, bo):
    common, Wq16, Wv16, Wo16, bqb16 = prep_weights(Wv, bv, Wq, bq, wa, Wo, bo)
    B = np.asarray(v).shape[0]
    return [
        dict(common, **prep_example(Wq16, Wv16, Wo16, bqb16, v[b], q[b]))
        for b in range(B)
    ]


def kernel(v, q, Wv, bv, Wq, bq, wa, ba, Wo, bo):
    from concourse.bass_utils import run_bass_kernel_spmd

    nc = get_nc()
    B = np.asarray(v).shape[0]
    assert B == N_CORES, f"expected batch {N_CORES}, got {B}"
    in_maps = make_in_maps(v, q, Wv, bv, Wq, bq, wa, Wo, bo)
    res = run_bass_kernel_spmd(nc, in_maps, core_ids=list(range(N_CORES)))
    fused = np.stack(
        [r["fusedT"].T.reshape(H_DIM) for r in res.results], axis=0
    )
    att = np.stack([unprep_att(r["attT16"]) for r in res.results], axis=0)
    att = att.reshape(B, H_OUT, NV * NQ, 1)
    return fused, att
